# revision 2
# baseline (speedup 1.0000x reference)
"""Trainium2 Bass kernel for nn_ContrastiveLoss (B=4096, F=256, T=0.1).

Strategy (8 NeuronCores, data parallel over the 2B=8192 rows of the combined
normalized matrix):
  - every core receives the full inputs, normalizes all 8192 rows to unit
    vectors (bf16), builds the transposed matrix cT [256, 8192] via PE
    transposes, and computes its 1024-row block of sim = (C @ C.T)/T
    fused with exp + row-sum accumulation (log-sum-exp without max
    subtraction: |s| <= 10 so exp is safely in fp32 range).
  - the diagonal is excluded analytically: d_i = ||c_i||^2 computed from the
    same bf16 values the matmul consumes, so exp(10*d_i) cancels the diagonal
    term of the accumulated exp row-sum on the host.
  - raw row-sums of s are never materialized: sum_{i in blk, all j} s_ij =
    (sum_{i in blk} c_i) . (sum_j c_j) / T, shipped as two column-sum vectors.
  - each core ships a [128, 24] f32 stats tile; the host finishes in float64:
    lse_i = log(E_i - exp(10 d_i)), neg = raw_excl - (2B-1) * sum(lse),
    loss = -mean(pos)/T + neg/(4B^2).
"""

import sys

sys.path.insert(0, "/opt/trn_rl_repo")

from contextlib import ExitStack  # noqa: E402

import numpy as np  # noqa: E402

import concourse.bass as bass  # noqa: E402
import concourse.mybir as mybir  # noqa: E402
import concourse.tile as tile  # noqa: E402
from concourse import bacc  # noqa: E402
from concourse.bass_utils import run_bass_kernel_spmd  # noqa: E402
from concourse.masks import make_identity  # noqa: E402

B = 4096
F = 256
TWO_B = 2 * B
N_CORES = 8
INV_T = 10.0  # 1 / temperature
EPS2 = 1e-14  # eps^2 for the norm clamp

F32 = mybir.dt.float32
BF16 = mybir.dt.bfloat16
U32 = mybir.dt.uint32

NT = 64  # 128-row tiles of the combined matrix
NBLK = 8  # 128-row tiles of this core's row block (1024 rows)
NPOS = 4  # 128-row tiles of this core's positive-pair slice (512 rows)
NSS = NT + NBLK + 2 * NPOS  # 80 columns of sum-of-squares / rsqrt values

# stats tile layout (columns)
S_E = 0  # 0:8   exp row-sums per row-tile (incl. diagonal term)
S_D = 8  # 8:16  d_i = ||c_i||^2 (bf16 values, fp32 sum) for own rows
S_POS = 16  # 16:20 positive-pair dot partial sums (fp32 path)
S_GB = 20  # 20:22 column sums of own 1024-row block of cT (per K-chunk)
S_GF = 22  # 22:24 column sums of all 8192 rows of cT (per K-chunk)
S_W = 24


def _build_kernel():
    nc = bacc.Bacc("TRN2", target_bir_lowering=False, debug=False, num_devices=N_CORES)

    first = nc.dram_tensor("first_transformed", [B, F], F32, kind="ExternalInput")
    second = nc.dram_tensor("second_transformed", [B, F], F32, kind="ExternalInput")
    blk = nc.dram_tensor("blk_raw", [NBLK * 128, F], F32, kind="ExternalInput")
    pos_a = nc.dram_tensor("pos_a", [NPOS * 128, F], F32, kind="ExternalInput")
    pos_b = nc.dram_tensor("pos_b", [NPOS * 128, F], F32, kind="ExternalInput")
    out = nc.dram_tensor("out", [128, S_W], F32, kind="ExternalOutput")

    with tile.TileContext(nc) as tc, ExitStack() as ctx:
        singles = ctx.enter_context(tc.tile_pool(name="singles", bufs=1))
        scr = ctx.enter_context(tc.tile_pool(name="scr", bufs=3))

        identity = singles.tile([128, 128], BF16)
        make_identity(nc, identity[:])

        stats = singles.tile([128, S_W], F32)

        # ---- load raw inputs -------------------------------------------------
        raw_f = singles.tile([128, 32, F], F32)
        raw_s = singles.tile([128, 32, F], F32)
        raw_blk = singles.tile([128, NBLK, F], F32)
        raw_pa = singles.tile([128, NPOS, F], F32)
        raw_pb = singles.tile([128, NPOS, F], F32)

        f_t = first.ap().rearrange("(t p) f -> p t f", p=128)
        s_t = second.ap().rearrange("(t p) f -> p t f", p=128)
        for j in range(4):
            nc.sync.dma_start(raw_f[:, 8 * j : 8 * (j + 1), :], f_t[:, 8 * j : 8 * (j + 1), :])
            nc.sync.dma_start(raw_s[:, 8 * j : 8 * (j + 1), :], s_t[:, 8 * j : 8 * (j + 1), :])
        nc.sync.dma_start(raw_blk[:], blk.ap().rearrange("(t p) f -> p t f", p=128))
        nc.sync.dma_start(raw_pa[:], pos_a.ap().rearrange("(t p) f -> p t f", p=128))
        nc.sync.dma_start(raw_pb[:], pos_b.ap().rearrange("(t p) f -> p t f", p=128))

        def raw_tile(t):
            if t < NT:
                return raw_f[:, t, :] if t < 32 else raw_s[:, t - 32, :]
            if t < NT + NBLK:
                return raw_blk[:, t - NT, :]
            if t < NT + NBLK + NPOS:
                return raw_pa[:, t - NT - NBLK, :]
            return raw_pb[:, t - NT - NBLK - NPOS, :]

        # ---- sum of squares per row -----------------------------------------
        ss = singles.tile([128, NSS], F32)
        for t in range(NSS):
            sq = scr.tile([128, F], F32, tag="sq")
            nc.vector.scalar_tensor_tensor(
                out=sq[:],
                in0=raw_tile(t),
                scalar=0.0,
                in1=raw_tile(t),
                op0=mybir.AluOpType.bypass,
                op1=mybir.AluOpType.mult,
                accum_out=ss[:, t : t + 1],
            )
        nc.vector.tensor_scalar_max(ss[:], ss[:], EPS2)

        # ---- rsqrt via bit trick + 3 Newton iterations (pure DVE) -----------
        magic = singles.tile([128, NSS], U32)
        nc.vector.memset(magic[:], 0x5F3759DF)
        one_u = singles.tile([128, 1], U32)
        nc.vector.memset(one_u[:], 1)
        halfbits = singles.tile([128, NSS], U32)
        nc.vector.tensor_scalar(
            halfbits[:], ss[:].bitcast(U32), one_u[:], None,
            mybir.AluOpType.logical_shift_right,
        )
        y = singles.tile([128, NSS], F32)
        nc.vector.tensor_tensor(y[:].bitcast(U32), magic[:], halfbits[:], mybir.AluOpType.subtract)
        for _ in range(3):
            t1 = scr.tile([128, NSS], F32, tag="nr")
            nc.vector.tensor_tensor(t1[:], y[:], y[:], mybir.AluOpType.mult)
            t2 = scr.tile([128, NSS], F32, tag="nr")
            nc.vector.scalar_tensor_tensor(
                out=t2[:], in0=t1[:], scalar=-0.5, in1=ss[:],
                op0=mybir.AluOpType.mult, op1=mybir.AluOpType.mult,
            )
            t3 = scr.tile([128, NSS], F32, tag="nr")
            nc.vector.tensor_scalar_add(t3[:], t2[:], 1.5)
            nc.vector.tensor_tensor(y[:], y[:], t3[:], mybir.AluOpType.mult)

        # ---- scale rows to unit norm ----------------------------------------
        scaled_f = singles.tile([128, 32, F], BF16)
        scaled_s = singles.tile([128, 32, F], BF16)
        scaled_blk = singles.tile([128, NBLK, F], BF16)
        scaled_pa = singles.tile([128, NPOS, F], F32)
        scaled_pb = singles.tile([128, NPOS, F], F32)

        def scaled_tile(t):
            if t < NT:
                return scaled_f[:, t, :] if t < 32 else scaled_s[:, t - 32, :]
            if t < NT + NBLK:
                return scaled_blk[:, t - NT, :]
            if t < NT + NBLK + NPOS:
                return scaled_pa[:, t - NT - NBLK, :]
            return scaled_pb[:, t - NT - NBLK - NPOS, :]

        for t in range(NSS):
            nc.vector.tensor_scalar_mul(scaled_tile(t), raw_tile(t), y[:, t : t + 1])

        # ---- positive-pair dots (fp32 path) and d_i (bf16 path) -------------
        for m in range(NPOS):
            sq = scr.tile([128, F], F32, tag="sq")
            nc.vector.scalar_tensor_tensor(
                out=sq[:], in0=scaled_pa[:, m, :], scalar=0.0, in1=scaled_pb[:, m, :],
                op0=mybir.AluOpType.bypass, op1=mybir.AluOpType.mult,
                accum_out=stats[:, S_POS + m : S_POS + m + 1],
            )
        for m in range(NBLK):
            sq = scr.tile([128, F], F32, tag="sq")
            nc.vector.scalar_tensor_tensor(
                out=sq[:], in0=scaled_blk[:, m, :], scalar=0.0, in1=scaled_blk[:, m, :],
                op0=mybir.AluOpType.bypass, op1=mybir.AluOpType.mult,
                accum_out=stats[:, S_D + m : S_D + m + 1],
            )

        # ---- transpose to cT / blkT via PE ----------------------------------
        cT = [singles.tile([128, TWO_B], BF16, name=f"cT{c}") for c in range(2)]
        blkT = [
            singles.tile([128, NBLK * 128], BF16, name=f"blkT{c}") for c in range(2)
        ]

        with tc.tile_pool(name="tp", bufs=2, space="PSUM") as tp:
            for c in range(2):
                for j in range(16):
                    pt = tp.tile([128, 512], BF16, tag="pt")
                    for q in range(4):
                        t = 4 * j + q
                        nc.tensor.transpose(
                            pt[:, 128 * q : 128 * (q + 1)],
                            scaled_tile(t)[:, 128 * c : 128 * (c + 1)],
                            identity[:],
                        )
                    nc.vector.tensor_copy(cT[c][:, 512 * j : 512 * (j + 1)], pt[:])
                for j in range(2):
                    pt = tp.tile([128, 512], BF16, tag="pt")
                    for q in range(4):
                        t = NT + 4 * j + q
                        nc.tensor.transpose(
                            pt[:, 128 * q : 128 * (q + 1)],
                            scaled_tile(t)[:, 128 * c : 128 * (c + 1)],
                            identity[:],
                        )
                    nc.vector.tensor_copy(blkT[c][:, 512 * j : 512 * (j + 1)], pt[:])

        # ---- column-sum vectors ---------------------------------------------
        for c in range(2):
            nc.vector.tensor_reduce(
                stats[:, S_GF + c : S_GF + c + 1], cT[c][:],
                mybir.AxisListType.X, mybir.AluOpType.add,
            )
            nc.vector.tensor_reduce(
                stats[:, S_GB + c : S_GB + c + 1], blkT[c][:],
                mybir.AxisListType.X, mybir.AluOpType.add,
            )

        # ---- main loop: sim row-block x exp + row-sum accumulation ----------
        e_parts = singles.tile([128, NBLK * 4], F32)
        with (
            tc.tile_pool(name="mm", bufs=2, space="PSUM") as mm,
            tc.tile_pool(name="escr", bufs=3) as escr,
        ):
            for m in range(NBLK):
                for g in range(4):
                    pt = mm.tile([128, 2048], F32, tag="mmt")
                    for h in range(4):
                        noff = 2048 * g + 512 * h
                        nc.tensor.matmul(
                            pt[:, 512 * h : 512 * (h + 1)],
                            blkT[0][:, 128 * m : 128 * (m + 1)],
                            cT[0][:, noff : noff + 512],
                            start=True, stop=False,
                        )
                        nc.tensor.matmul(
                            pt[:, 512 * h : 512 * (h + 1)],
                            blkT[1][:, 128 * m : 128 * (m + 1)],
                            cT[1][:, noff : noff + 512],
                            start=False, stop=True,
                        )
                    et = escr.tile([128, 2048], BF16, tag="et")
                    idx = 4 * m + g
                    nc.scalar.activation(
                        et[:], pt[:], mybir.ActivationFunctionType.Exp,
                        bias=0.0, scale=INV_T,
                        accum_out=e_parts[:, idx : idx + 1],
                    )

        for m in range(NBLK):
            nc.vector.tensor_reduce(
                stats[:, S_E + m : S_E + m + 1], e_parts[:, 4 * m : 4 * (m + 1)],
                mybir.AxisListType.X, mybir.AluOpType.add,
            )

        nc.sync.dma_start(out.ap(), stats[:])

    nc.compile()
    return nc


_NC_CACHE = None


def _get_nc():
    global _NC_CACHE
    if _NC_CACHE is None:
        _NC_CACHE = _build_kernel()
    return _NC_CACHE


def make_in_maps(first, second):
    f = np.ascontiguousarray(first, dtype=np.float32)
    s = np.ascontiguousarray(second, dtype=np.float32)
    in_maps = []
    for k in range(N_CORES):
        if k < 4:
            blk = f[1024 * k : 1024 * (k + 1)]
        else:
            blk = s[1024 * (k - 4) : 1024 * (k - 3)]
        in_maps.append(
            {
                "first_transformed": f,
                "second_transformed": s,
                "blk_raw": np.ascontiguousarray(blk),
                "pos_a": np.ascontiguousarray(f[512 * k : 512 * (k + 1)]),
                "pos_b": np.ascontiguousarray(s[512 * k : 512 * (k + 1)]),
            }
        )
    return in_maps


def combine_outputs(stats_per_core):
    """stats_per_core: list of 8 [128, 24] f32 arrays -> scalar loss (f32)."""
    lse_tot = 0.0
    raw_excl_tot = 0.0
    pos_tot = 0.0
    for st in stats_per_core:
        st = np.asarray(st, dtype=np.float64)
        e_sum = st[:, S_E : S_E + 8]
        d = st[:, S_D : S_D + 8]
        pos = st[:, S_POS : S_POS + 4]
        gb = st[:, S_GB : S_GB + 2]
        gf = st[:, S_GF : S_GF + 2]
        e_excl = e_sum - np.exp(INV_T * d)
        lse_tot += np.log(e_excl).sum()
        raw_excl_tot += (np.sum(gb * gf) - d.sum()) * INV_T
        pos_tot += pos.sum()
    neg = raw_excl_tot - (TWO_B - 1) * lse_tot
    loss = -pos_tot * INV_T / B + neg / (4.0 * B * B)
    return np.asarray(loss, dtype=np.float32)


def kernel(first_transformed, second_transformed):
    nc = _get_nc()
    in_maps = make_in_maps(first_transformed, second_transformed)
    res = run_bass_kernel_spmd(nc, in_maps, core_ids=list(range(N_CORES)))
    return combine_outputs([res.results[i]["out"] for i in range(N_CORES)])


# revision 4
# speedup vs baseline: 442.1157x; 442.1157x over previous
"""Trainium2 Bass kernel for nn_ContrastiveLoss (B=4096, F=256, T=0.1).

Strategy (8 NeuronCores, data parallel over the 2B=8192 rows of the combined
normalized matrix):
  - every core receives the full inputs, normalizes all 8192 rows to unit
    vectors (bf16), builds the transposed matrix cT [256, 8192] via PE
    transposes, and computes its 1024-row block of sim = (C @ C.T)/T
    fused with exp + row-sum accumulation (log-sum-exp without max
    subtraction: |s| <= 10 so exp is safely in fp32 range).
  - the diagonal is excluded analytically: d_i = ||c_i||^2 computed from the
    same bf16 values the matmul consumes, so exp(10*d_i) cancels the diagonal
    term of the accumulated exp row-sum on the host.
  - raw row-sums of s are never materialized: sum_{i in blk, all j} s_ij =
    (sum_{i in blk} c_i) . (sum_j c_j) / T, shipped as two column-sum vectors.
  - each core ships a [128, 24] f32 stats tile; the host finishes in float64:
    lse_i = log(E_i - exp(10 d_i)), neg = raw_excl - (2B-1) * sum(lse),
    loss = -mean(pos)/T + neg/(4B^2).
"""

import sys

sys.path.insert(0, "/opt/trn_rl_repo")

from contextlib import ExitStack  # noqa: E402

import numpy as np  # noqa: E402

import concourse.bass as bass  # noqa: E402
import concourse.mybir as mybir  # noqa: E402
import concourse.tile as tile  # noqa: E402
from concourse import bacc  # noqa: E402
from concourse.bass_utils import run_bass_kernel_spmd  # noqa: E402
from concourse.masks import make_identity  # noqa: E402

B = 4096
F = 256
TWO_B = 2 * B
N_CORES = 8
INV_T = 10.0  # 1 / temperature
EPS2 = 1e-14  # eps^2 for the norm clamp

F32 = mybir.dt.float32
BF16 = mybir.dt.bfloat16
U32 = mybir.dt.uint32

NT = 64  # 128-row tiles of the combined matrix
NBLK = 8  # 128-row tiles of this core's row block (1024 rows)
NPOS = 4  # 128-row tiles of this core's positive-pair slice (512 rows)
NSS = NT + NBLK + 2 * NPOS  # 80 columns of sum-of-squares / rsqrt values

# stats tile layout (columns)
S_E = 0  # 0:8   exp row-sums per row-tile (incl. diagonal term)
S_D = 8  # 8:16  d_i = ||c_i||^2 (bf16 values, fp32 sum) for own rows
S_POS = 16  # 16:20 positive-pair dot partial sums (fp32 path)
S_GB = 20  # 20:22 column sums of own 1024-row block of cT (per K-chunk)
S_GF = 22  # 22:24 column sums of all 8192 rows of cT (per K-chunk)
S_W = 24


def _build_kernel(loop_n=None):
    """loop_n: if set, wrap the whole body in a device-side For_i loop that
    executes it loop_n times (used only for timing measurements)."""
    nc = bacc.Bacc("TRN2", target_bir_lowering=False, debug=False, num_devices=N_CORES)

    first = nc.dram_tensor("first_transformed", [B, F], F32, kind="ExternalInput")
    second = nc.dram_tensor("second_transformed", [B, F], F32, kind="ExternalInput")
    blk = nc.dram_tensor("blk_raw", [NBLK * 128, F], F32, kind="ExternalInput")
    pos_a = nc.dram_tensor("pos_a", [NPOS * 128, F], F32, kind="ExternalInput")
    pos_b = nc.dram_tensor("pos_b", [NPOS * 128, F], F32, kind="ExternalInput")
    out = nc.dram_tensor("out", [128, S_W], F32, kind="ExternalOutput")

    with tile.TileContext(nc) as tc, ExitStack() as octx:
        if loop_n is not None:
            octx.enter_context(tc.For_i(0, loop_n, 1))
        _emit_body(nc, tc, first, second, blk, pos_a, pos_b, out)

    nc.compile()
    return nc


def _emit_body(nc, tc, first, second, blk, pos_a, pos_b, out):
    with ExitStack() as ctx:
        singles = ctx.enter_context(tc.tile_pool(name="singles", bufs=1))
        scr = ctx.enter_context(tc.tile_pool(name="scr", bufs=3))

        identity = singles.tile([128, 128], BF16)
        make_identity(nc, identity[:])

        stats = singles.tile([128, S_W], F32)

        # ---- load raw inputs -------------------------------------------------
        raw_f = singles.tile([128, 32, F], F32)
        raw_s = singles.tile([128, 32, F], F32)
        raw_blk = singles.tile([128, NBLK, F], F32)
        raw_pa = singles.tile([128, NPOS, F], F32)
        raw_pb = singles.tile([128, NPOS, F], F32)

        f_t = first.ap().rearrange("(t p) f -> p t f", p=128)
        s_t = second.ap().rearrange("(t p) f -> p t f", p=128)
        for j in range(4):
            nc.sync.dma_start(raw_f[:, 8 * j : 8 * (j + 1), :], f_t[:, 8 * j : 8 * (j + 1), :])
            nc.sync.dma_start(raw_s[:, 8 * j : 8 * (j + 1), :], s_t[:, 8 * j : 8 * (j + 1), :])
        nc.sync.dma_start(raw_blk[:], blk.ap().rearrange("(t p) f -> p t f", p=128))
        nc.sync.dma_start(raw_pa[:], pos_a.ap().rearrange("(t p) f -> p t f", p=128))
        nc.sync.dma_start(raw_pb[:], pos_b.ap().rearrange("(t p) f -> p t f", p=128))

        def raw_tile(t):
            if t < NT:
                return raw_f[:, t, :] if t < 32 else raw_s[:, t - 32, :]
            if t < NT + NBLK:
                return raw_blk[:, t - NT, :]
            if t < NT + NBLK + NPOS:
                return raw_pa[:, t - NT - NBLK, :]
            return raw_pb[:, t - NT - NBLK - NPOS, :]

        # ---- sum of squares per row -----------------------------------------
        ss = singles.tile([128, NSS], F32)
        for t in range(NSS):
            sq = scr.tile([128, F], F32, tag="sq")
            nc.vector.scalar_tensor_tensor(
                out=sq[:],
                in0=raw_tile(t),
                scalar=0.0,
                in1=raw_tile(t),
                op0=mybir.AluOpType.bypass,
                op1=mybir.AluOpType.mult,
                accum_out=ss[:, t : t + 1],
            )
        nc.vector.tensor_scalar_max(ss[:], ss[:], EPS2)

        # ---- rsqrt via bit trick + 3 Newton iterations (pure DVE) -----------
        magic = singles.tile([128, NSS], U32)
        nc.vector.memset(magic[:], 0x5F3759DF)
        one_u = singles.tile([128, 1], U32)
        nc.vector.memset(one_u[:], 1)
        halfbits = singles.tile([128, NSS], U32)
        nc.vector.tensor_scalar(
            halfbits[:], ss[:].bitcast(U32), one_u[:], None,
            mybir.AluOpType.logical_shift_right,
        )
        y = singles.tile([128, NSS], F32)
        nc.vector.tensor_tensor(y[:].bitcast(U32), magic[:], halfbits[:], mybir.AluOpType.subtract)
        for _ in range(3):
            t1 = scr.tile([128, NSS], F32, tag="nr")
            nc.vector.tensor_tensor(t1[:], y[:], y[:], mybir.AluOpType.mult)
            t2 = scr.tile([128, NSS], F32, tag="nr")
            nc.vector.scalar_tensor_tensor(
                out=t2[:], in0=t1[:], scalar=-0.5, in1=ss[:],
                op0=mybir.AluOpType.mult, op1=mybir.AluOpType.mult,
            )
            t3 = scr.tile([128, NSS], F32, tag="nr")
            nc.vector.tensor_scalar_add(t3[:], t2[:], 1.5)
            nc.vector.tensor_tensor(y[:], y[:], t3[:], mybir.AluOpType.mult)

        # ---- scale rows to unit norm ----------------------------------------
        scaled_f = singles.tile([128, 32, F], BF16)
        scaled_s = singles.tile([128, 32, F], BF16)
        scaled_blk = singles.tile([128, NBLK, F], BF16)
        scaled_pa = singles.tile([128, NPOS, F], F32)
        scaled_pb = singles.tile([128, NPOS, F], F32)

        def scaled_tile(t):
            if t < NT:
                return scaled_f[:, t, :] if t < 32 else scaled_s[:, t - 32, :]
            if t < NT + NBLK:
                return scaled_blk[:, t - NT, :]
            if t < NT + NBLK + NPOS:
                return scaled_pa[:, t - NT - NBLK, :]
            return scaled_pb[:, t - NT - NBLK - NPOS, :]

        for t in range(NSS):
            nc.vector.tensor_scalar_mul(scaled_tile(t), raw_tile(t), y[:, t : t + 1])

        # ---- positive-pair dots (fp32 path) and d_i (bf16 path) -------------
        for m in range(NPOS):
            sq = scr.tile([128, F], F32, tag="sq")
            nc.vector.scalar_tensor_tensor(
                out=sq[:], in0=scaled_pa[:, m, :], scalar=0.0, in1=scaled_pb[:, m, :],
                op0=mybir.AluOpType.bypass, op1=mybir.AluOpType.mult,
                accum_out=stats[:, S_POS + m : S_POS + m + 1],
            )
        for m in range(NBLK):
            sq = scr.tile([128, F], F32, tag="sq")
            nc.vector.scalar_tensor_tensor(
                out=sq[:], in0=scaled_blk[:, m, :], scalar=0.0, in1=scaled_blk[:, m, :],
                op0=mybir.AluOpType.bypass, op1=mybir.AluOpType.mult,
                accum_out=stats[:, S_D + m : S_D + m + 1],
            )

        # ---- transpose to cT / blkT via PE ----------------------------------
        cT = [singles.tile([128, TWO_B], BF16, name=f"cT{c}") for c in range(2)]
        blkT = [
            singles.tile([128, NBLK * 128], BF16, name=f"blkT{c}") for c in range(2)
        ]

        with tc.tile_pool(name="tp", bufs=2, space="PSUM") as tp:
            for c in range(2):
                for j in range(16):
                    pt = tp.tile([128, 512], BF16, tag="pt")
                    for q in range(4):
                        t = 4 * j + q
                        nc.tensor.transpose(
                            pt[:, 128 * q : 128 * (q + 1)],
                            scaled_tile(t)[:, 128 * c : 128 * (c + 1)],
                            identity[:],
                        )
                    nc.vector.tensor_copy(cT[c][:, 512 * j : 512 * (j + 1)], pt[:])
                for j in range(2):
                    pt = tp.tile([128, 512], BF16, tag="pt")
                    for q in range(4):
                        t = NT + 4 * j + q
                        nc.tensor.transpose(
                            pt[:, 128 * q : 128 * (q + 1)],
                            scaled_tile(t)[:, 128 * c : 128 * (c + 1)],
                            identity[:],
                        )
                    nc.vector.tensor_copy(blkT[c][:, 512 * j : 512 * (j + 1)], pt[:])

        # ---- column-sum vectors ---------------------------------------------
        for c in range(2):
            nc.vector.tensor_reduce(
                stats[:, S_GF + c : S_GF + c + 1], cT[c][:],
                mybir.AxisListType.X, mybir.AluOpType.add,
            )
            nc.vector.tensor_reduce(
                stats[:, S_GB + c : S_GB + c + 1], blkT[c][:],
                mybir.AxisListType.X, mybir.AluOpType.add,
            )

        # ---- main loop: sim row-block x exp + row-sum accumulation ----------
        e_parts = singles.tile([128, NBLK * 4], F32)
        with (
            tc.tile_pool(name="mm", bufs=2, space="PSUM") as mm,
            tc.tile_pool(name="escr", bufs=3) as escr,
        ):
            for m in range(NBLK):
                for g in range(4):
                    pt = mm.tile([128, 2048], F32, tag="mmt")
                    for h in range(4):
                        noff = 2048 * g + 512 * h
                        nc.tensor.matmul(
                            pt[:, 512 * h : 512 * (h + 1)],
                            blkT[0][:, 128 * m : 128 * (m + 1)],
                            cT[0][:, noff : noff + 512],
                            start=True, stop=False,
                        )
                        nc.tensor.matmul(
                            pt[:, 512 * h : 512 * (h + 1)],
                            blkT[1][:, 128 * m : 128 * (m + 1)],
                            cT[1][:, noff : noff + 512],
                            start=False, stop=True,
                        )
                    et = escr.tile([128, 2048], BF16, tag="et")
                    idx = 4 * m + g
                    nc.scalar.activation(
                        et[:], pt[:], mybir.ActivationFunctionType.Exp,
                        bias=0.0, scale=INV_T,
                        accum_out=e_parts[:, idx : idx + 1],
                    )

        for m in range(NBLK):
            nc.vector.tensor_reduce(
                stats[:, S_E + m : S_E + m + 1], e_parts[:, 4 * m : 4 * (m + 1)],
                mybir.AxisListType.X, mybir.AluOpType.add,
            )

        nc.sync.dma_start(out.ap(), stats[:])


_NC_CACHE = None


def _get_nc():
    global _NC_CACHE
    if _NC_CACHE is None:
        _NC_CACHE = _build_kernel()
    return _NC_CACHE


def make_in_maps(first, second):
    f = np.ascontiguousarray(first, dtype=np.float32)
    s = np.ascontiguousarray(second, dtype=np.float32)
    in_maps = []
    for k in range(N_CORES):
        if k < 4:
            blk = f[1024 * k : 1024 * (k + 1)]
        else:
            blk = s[1024 * (k - 4) : 1024 * (k - 3)]
        in_maps.append(
            {
                "first_transformed": f,
                "second_transformed": s,
                "blk_raw": np.ascontiguousarray(blk),
                "pos_a": np.ascontiguousarray(f[512 * k : 512 * (k + 1)]),
                "pos_b": np.ascontiguousarray(s[512 * k : 512 * (k + 1)]),
            }
        )
    return in_maps


def combine_outputs(stats_per_core):
    """stats_per_core: list of 8 [128, 24] f32 arrays -> scalar loss (f32)."""
    lse_tot = 0.0
    raw_excl_tot = 0.0
    pos_tot = 0.0
    for st in stats_per_core:
        st = np.asarray(st, dtype=np.float64)
        e_sum = st[:, S_E : S_E + 8]
        d = st[:, S_D : S_D + 8]
        pos = st[:, S_POS : S_POS + 4]
        gb = st[:, S_GB : S_GB + 2]
        gf = st[:, S_GF : S_GF + 2]
        e_excl = e_sum - np.exp(INV_T * d)
        lse_tot += np.log(e_excl).sum()
        raw_excl_tot += (np.sum(gb * gf) - d.sum()) * INV_T
        pos_tot += pos.sum()
    neg = raw_excl_tot - (TWO_B - 1) * lse_tot
    loss = -pos_tot * INV_T / B + neg / (4.0 * B * B)
    return np.asarray(loss, dtype=np.float32)


def kernel(first_transformed, second_transformed):
    nc = _get_nc()
    in_maps = make_in_maps(first_transformed, second_transformed)
    res = run_bass_kernel_spmd(nc, in_maps, core_ids=list(range(N_CORES)))
    return combine_outputs([res.results[i]["out"] for i in range(N_CORES)])


# revision 15
# speedup vs baseline: 645.2590x; 1.4595x over previous
"""Trainium2 Bass kernel for nn_ContrastiveLoss (B=4096, F=256, T=0.1).

Strategy (8 NeuronCores, data parallel over the 2B=8192 rows of the combined
normalized matrix):
  - every core receives the full inputs, normalizes all 8192 rows to unit
    vectors (bf16), builds the transposed matrix cT [256, 8192] via DMA xbar
    transposes, and computes its 1024-row block of sim = (C @ C.T)/T fused
    with exp + row-sum accumulation (log-sum-exp without max subtraction:
    |s| <= 10 so exp is safely in fp32 range).
  - the work is pipelined in 4 column groups of 2048: normalize group g,
    DMA-transpose it into cT, matmul+exp against it while group g+1 loads.
  - the diagonal is excluded analytically: d_i = ||c_i||^2 computed from the
    same bf16 values the matmul consumes, so exp(10*d_i) cancels the diagonal
    term of the accumulated exp row-sum on the host.
  - raw row-sums of s are never materialized: sum_{i in blk, all j} s_ij =
    (sum_{i in blk} c_i) . (sum_j c_j) / T, shipped as two column-sum vectors.
  - each core ships a [128, 24] f32 stats tile; the host finishes in float64:
    lse_i = log(E_i - exp(10 d_i)), neg = raw_excl - (2B-1) * sum(lse),
    loss = -mean(pos)/T + neg/(4B^2).
"""

import sys

sys.path.insert(0, "/opt/trn_rl_repo")

from contextlib import ExitStack  # noqa: E402

import numpy as np  # noqa: E402

import concourse.bass as bass  # noqa: E402
import concourse.mybir as mybir  # noqa: E402
import concourse.tile as tile  # noqa: E402
from concourse import bacc  # noqa: E402
from concourse.bass_utils import run_bass_kernel_spmd  # noqa: E402

B = 4096
F = 256
TWO_B = 2 * B
N_CORES = 8
INV_T = 10.0  # 1 / temperature
EPS2 = 1e-14  # eps^2 for the norm clamp

F32 = mybir.dt.float32
BF16 = mybir.dt.bfloat16
U32 = mybir.dt.uint32
OP = mybir.AluOpType

NT = 64  # 128-row tiles of the combined matrix
NBLK = 8  # 128-row tiles of this core's row block (1024 rows)
NPOS = 4  # 128-row tiles of this core's positive-pair slice (512 rows)
NSS = NT + NBLK + 2 * NPOS  # 80 row-tile slots: 64 combined, 8 blk, 4+4 pos

# stats tile layout (columns)
S_E = 0  # 0:8   exp row-sums per row-tile (incl. diagonal term)
S_D = 8  # 8:16  d_i = ||c_i||^2 (bf16 values, fp32 sum) for own rows
S_POS = 16  # 16:20 positive-pair dot partial sums (fp32 path)
S_GB = 20  # 20:22 column sums of own 1024-row block of cT (per K-chunk)
S_GF = 22  # 22:24 column sums of all 8192 rows of cT (per K-chunk)
S_W = 24


def _build_kernel(loop_n=None):
    """loop_n: if set, wrap the whole body in a device-side For_i loop that
    executes it loop_n times (used only for timing measurements)."""
    nc = bacc.Bacc("TRN2", target_bir_lowering=False, debug=False, num_devices=N_CORES)

    first = nc.dram_tensor("first_transformed", [B, F], F32, kind="ExternalInput")
    second = nc.dram_tensor("second_transformed", [B, F], F32, kind="ExternalInput")
    blk = nc.dram_tensor("blk_raw", [NBLK * 128, F], F32, kind="ExternalInput")
    pos_a = nc.dram_tensor("pos_a", [NPOS * 128, F], F32, kind="ExternalInput")
    pos_b = nc.dram_tensor("pos_b", [NPOS * 128, F], F32, kind="ExternalInput")
    out = nc.dram_tensor("out", [128, S_W], F32, kind="ExternalOutput")

    with tile.TileContext(nc) as tc, ExitStack() as octx:
        if loop_n is not None:
            octx.enter_context(tc.For_i(0, loop_n, 1))
        _emit_body(nc, tc, first, second, blk, pos_a, pos_b, out)

    nc.compile()
    return nc


def _emit_body(nc, tc, first, second, blk, pos_a, pos_b, out):
    with ExitStack() as ctx:
        singles = ctx.enter_context(tc.tile_pool(name="singles", bufs=1))
        scr = ctx.enter_context(tc.tile_pool(name="scr", bufs=4))

        stats = singles.tile([128, S_W], F32)

        # persistent SBUF tensors
        raw_all = singles.tile([128, NSS, F], F32)  # 40KB/partition
        # chunk-major bf16 normalized rows: [partition, K-chunk, row-tile, 128]
        # so one xbar DMA can transpose a whole 16-tile group per chunk
        scaled_cmb = singles.tile([128, 2, NT, 128], BF16)
        scaled_blk = singles.tile([128, 2, NBLK, 128], BF16)
        scaled_pa = singles.tile([128, NPOS, F], F32)
        scaled_pb = singles.tile([128, NPOS, F], F32)
        cT = [singles.tile([128, TWO_B], BF16, name=f"cT{c}") for c in range(2)]
        blkT = [
            singles.tile([128, NBLK * 128], BF16, name=f"blkT{c}") for c in range(2)
        ]
        ss = singles.tile([128, NSS], F32)
        y = singles.tile([128, NSS], F32)
        e_parts = singles.tile([128, NBLK * 4], F32)
        # rsqrt seed constant 0x5f3759df held as a float VALUE: the classic
        # bit trick is done in f32 arithmetic (bits are ~2^30, f32 rounding of
        # the bit pattern perturbs the seed by ~1e-5 rel — Newton absorbs it)
        magicf = singles.tile([128, NSS], F32)
        nc.vector.memset(magicf[:], float(0x5F3759DF))

        def scaled_tile(t):
            """[128, 2, 128] (or [128, F]) view of the normalized row-tile t."""
            if t < NT:
                return scaled_cmb[:, :, t, :]
            if t < NT + NBLK:
                return scaled_blk[:, :, t - NT, :]
            if t < NT + NBLK + NPOS:
                return scaled_pa[:, t - NT - NBLK, :]
            return scaled_pb[:, t - NT - NBLK - NPOS, :]

        # ---- DMA loads (SP HWDGE ring, program order = FIFO order) ----------
        nc.sync.dma_start(
            raw_all[:, NT : NT + NBLK, :], blk.ap().rearrange("(t p) f -> p t f", p=128)
        )
        f_t = first.ap().rearrange("(t p) f -> p t f", p=128)
        s_t = second.ap().rearrange("(t p) f -> p t f", p=128)
        for g in range(4):
            src = f_t if g < 2 else s_t
            o = (g % 2) * 16
            nc.sync.dma_start(raw_all[:, 16 * g : 16 * (g + 1), :], src[:, o : o + 16, :])
        nc.sync.dma_start(
            raw_all[:, NT + NBLK : NT + NBLK + NPOS, :],
            pos_a.ap().rearrange("(t p) f -> p t f", p=128),
        )
        nc.sync.dma_start(
            raw_all[:, NT + NBLK + NPOS : NSS, :],
            pos_b.ap().rearrange("(t p) f -> p t f", p=128),
        )

        # ---- helpers --------------------------------------------------------
        def norm_group(t0, n):
            """sum-of-squares + rsqrt (Newton) + scale for row-tiles [t0, t0+n).
            2-input/elementwise work is split between DVE and GpSimd."""
            for i in range(n):
                t = t0 + i
                sq = scr.tile([128, F], F32, tag="sq")
                nc.vector.scalar_tensor_tensor(
                    out=sq[:],
                    in0=raw_all[:, t, :],
                    scalar=0.0,
                    in1=raw_all[:, t, :],
                    op0=OP.bypass,
                    op1=OP.mult,
                    accum_out=ss[:, t : t + 1],
                )
            sl = slice(t0, t0 + n)
            nc.vector.tensor_scalar_max(ss[:, sl], ss[:, sl], EPS2)
            bits_f = scr.tile([128, n], F32, tag="hb")
            nc.vector.tensor_copy(bits_f[:], ss[:, sl].bitcast(U32))  # uint -> f32
            seed_f = scr.tile([128, n], F32, tag="sf")
            nc.vector.scalar_tensor_tensor(
                out=seed_f[:], in0=bits_f[:], scalar=-0.5, in1=magicf[:, :n],
                op0=OP.mult, op1=OP.add,
            )
            nc.vector.tensor_copy(y[:, sl].bitcast(U32), seed_f[:])  # f32 -> uint
            for _ in range(3):
                t1 = scr.tile([128, n], F32, tag="nr")
                nc.vector.tensor_tensor(t1[:], y[:, sl], y[:, sl], OP.mult)
                t2 = scr.tile([128, n], F32, tag="nr")
                nc.vector.scalar_tensor_tensor(
                    out=t2[:], in0=t1[:], scalar=-0.5, in1=ss[:, sl],
                    op0=OP.mult, op1=OP.mult,
                )
                t3 = scr.tile([128, n], F32, tag="nr")
                nc.vector.tensor_scalar_add(t3[:], t2[:], 1.5)
                nc.vector.tensor_tensor(y[:, sl], y[:, sl], t3[:], OP.mult)
            for i in range(n):
                t = t0 + i
                dst = scaled_tile(t)
                src = raw_all[:, t, :]
                if t < NT + NBLK:  # 3D chunk-major destination
                    src = src.rearrange("p (c f) -> p c f", c=2)
                nc.vector.tensor_scalar_mul(dst, src, y[:, t : t + 1])

        def transpose_group(src3d, dst, dst_off, n):
            """xbar-transpose n contiguous chunk-major row-tiles into dst[c]
            columns [dst_off, dst_off + 128n) — one DMA per K-chunk."""
            for c in range(2):
                nc.sync.dma_start_transpose(
                    out=dst[c][:, dst_off : dst_off + 128 * n].rearrange(
                        "p (t m) -> p t m", m=128
                    ),
                    in_=src3d[:, c, :, :],
                )

        # ---- own row block first (needed by every matmul) -------------------
        norm_group(NT, NBLK)
        transpose_group(scaled_blk, blkT, 0, NBLK)

        # ---- pipelined main loop over 4 column groups of 2048 ---------------
        mm = ctx.enter_context(tc.tile_pool(name="mm", bufs=2, space="PSUM"))
        escr = ctx.enter_context(tc.tile_pool(name="escr", bufs=3))

        gparts = singles.tile([128, 4, 2], F32)

        for g in range(4):
            norm_group(16 * g, 16)
            transpose_group(scaled_cmb[:, :, 16 * g : 16 * (g + 1), :], cT, 2048 * g, 16)
            for c in range(2):
                nc.vector.tensor_reduce(
                    gparts[:, g, c : c + 1], cT[c][:, 2048 * g : 2048 * (g + 1)],
                    mybir.AxisListType.X, OP.add,
                )
            if g == 0:
                # independent side work, scheduled into the main-loop shadow
                norm_group(NT + NBLK, 2 * NPOS)
                for m in range(NPOS):
                    sq = scr.tile([128, F], F32, tag="sq")
                    nc.vector.scalar_tensor_tensor(
                        out=sq[:], in0=scaled_pa[:, m, :], scalar=0.0,
                        in1=scaled_pb[:, m, :],
                        op0=OP.bypass, op1=OP.mult,
                        accum_out=stats[:, S_POS + m : S_POS + m + 1],
                    )
                for m in range(NBLK):
                    sq = scr.tile([128, 2, 128], F32, tag="sqd")
                    nc.vector.scalar_tensor_tensor(
                        out=sq[:], in0=scaled_blk[:, :, m, :], scalar=0.0,
                        in1=scaled_blk[:, :, m, :],
                        op0=OP.bypass, op1=OP.mult,
                        accum_out=stats[:, S_D + m : S_D + m + 1],
                    )
            for m in range(NBLK):
                pt = mm.tile([128, 2048], F32, tag="mmt")
                for h in range(4):
                    noff = 2048 * g + 512 * h
                    nc.tensor.matmul(
                        pt[:, 512 * h : 512 * (h + 1)],
                        blkT[0][:, 128 * m : 128 * (m + 1)],
                        cT[0][:, noff : noff + 512],
                        start=True, stop=False,
                    )
                for h in range(4):
                    noff = 2048 * g + 512 * h
                    nc.tensor.matmul(
                        pt[:, 512 * h : 512 * (h + 1)],
                        blkT[1][:, 128 * m : 128 * (m + 1)],
                        cT[1][:, noff : noff + 512],
                        start=False, stop=True,
                    )
                et = escr.tile([128, 2048], BF16, tag="et")
                idx = 4 * m + g
                nc.scalar.activation(
                    et[:], pt[:], mybir.ActivationFunctionType.Exp,
                    bias=0.0, scale=INV_T,
                    accum_out=e_parts[:, idx : idx + 1],
                )

        # ---- column-sum vectors ---------------------------------------------
        for c in range(2):
            nc.vector.tensor_reduce(
                stats[:, S_GF + c : S_GF + c + 1], gparts[:, :, c],
                mybir.AxisListType.X, OP.add,
            )
            nc.vector.tensor_reduce(
                stats[:, S_GB + c : S_GB + c + 1], blkT[c][:],
                mybir.AxisListType.X, OP.add,
            )

        for m in range(NBLK):
            nc.vector.tensor_reduce(
                stats[:, S_E + m : S_E + m + 1], e_parts[:, 4 * m : 4 * (m + 1)],
                mybir.AxisListType.X, OP.add,
            )

        nc.sync.dma_start(out.ap(), stats[:])


_NC_CACHE = None


def _get_nc():
    global _NC_CACHE
    if _NC_CACHE is None:
        _NC_CACHE = _build_kernel()
    return _NC_CACHE


def make_in_maps(first, second):
    f = np.ascontiguousarray(first, dtype=np.float32)
    s = np.ascontiguousarray(second, dtype=np.float32)
    in_maps = []
    for k in range(N_CORES):
        if k < 4:
            blk = f[1024 * k : 1024 * (k + 1)]
        else:
            blk = s[1024 * (k - 4) : 1024 * (k - 3)]
        in_maps.append(
            {
                "first_transformed": f,
                "second_transformed": s,
                "blk_raw": np.ascontiguousarray(blk),
                "pos_a": np.ascontiguousarray(f[512 * k : 512 * (k + 1)]),
                "pos_b": np.ascontiguousarray(s[512 * k : 512 * (k + 1)]),
            }
        )
    return in_maps


def combine_outputs(stats_per_core):
    """stats_per_core: list of 8 [128, 24] f32 arrays -> scalar loss (f32)."""
    lse_tot = 0.0
    raw_excl_tot = 0.0
    pos_tot = 0.0
    for st in stats_per_core:
        st = np.asarray(st, dtype=np.float64)
        e_sum = st[:, S_E : S_E + 8]
        d = st[:, S_D : S_D + 8]
        pos = st[:, S_POS : S_POS + 4]
        gb = st[:, S_GB : S_GB + 2]
        gf = st[:, S_GF : S_GF + 2]
        e_excl = e_sum - np.exp(INV_T * d)
        lse_tot += np.log(e_excl).sum()
        raw_excl_tot += (np.sum(gb * gf) - d.sum()) * INV_T
        pos_tot += pos.sum()
    neg = raw_excl_tot - (TWO_B - 1) * lse_tot
    loss = -pos_tot * INV_T / B + neg / (4.0 * B * B)
    return np.asarray(loss, dtype=np.float32)


def kernel(first_transformed, second_transformed):
    nc = _get_nc()
    in_maps = make_in_maps(first_transformed, second_transformed)
    res = run_bass_kernel_spmd(nc, in_maps, core_ids=list(range(N_CORES)))
    return combine_outputs([res.results[i]["out"] for i in range(N_CORES)])
